# revision 1
# baseline (speedup 1.0000x reference)
"""CKConv Trainium2 kernel.

Math (derived from the reference):
  out[b,o,l] = sum_i sum_{d=0}^{l} g[o,i,d] * x[b,i,l-d] + conv_bias[o]
  g[o,i,d]   = k_full[o,i,2047-d],  k_full = w3 @ h2 + b3
  h2 = sin(30*(w2 @ h1 + b2)), h1 = sin(30*(w1 @ t + b1)), t = linspace(-1,1,L)
  Feeding tr = -t (= reversed t) gives h2r/k_rev with k_rev[:, d] = k_full[:, 2047-d],
  so g[o,i,d] = k_rev[16*o+i, d].

Mapping (per core, data-parallel over batch b):
  - XS bank [128, 16*2560] fp16: XS[d'', 2560*i + c] = x[b,i, c-511-d''] (0 outside),
    built by one seed DMA of host-padded x + 7 log-doubling shift DMAs.
  - SIREN computed on device in fp16 matmuls (t and 30*w1 split hi/lo for accuracy)
    with fp32 range reduction (magic-number round) before the ACT Sin LUT
    (LUT domain is [-pi, pi]).
  - L3 produces K_revT[t][d'', 32*i+o] = k_rev[16*o+i, 128*t+d''] directly:
    16 matmuls lhsT=H2flat[:,128t:+128] ([33,128], ones row for b3), rhs=W3T [33,512].
  - Conv: 640 matmuls [K=128, M=32, N=512] fp16: for (p,t,i):
      psum[g] += K16[:, 512t+32i:+32].T @ XS[:, 2560i + 511 + 512p - 128t : +512]
    spread over 4 PE column groups (tile_position) with per-(p,g) psum accumulators.
  - Partials [128, 2048] fp32 -> HBM; host sums the 4 group partials + conv_bias.
"""
import numpy as np

OMEGA0 = 30.0
CIN, COUT, HID = 16, 32, 32
B, L = 8, 2048
PAD = 511          # left zero pad inside each XS row block
XSW = 2560         # per-i XS row width: PAD + L + 1
PI = float(np.pi)
TWO_PI = float(2 * np.pi)
MAGIC = 12582912.0  # 1.5 * 2**23, fp32 round-to-nearest trick
INV_2PI = float(1.0 / (2 * np.pi))

_COMPILED = {}
_KERNEL_OPTS = {"trace": False, "last_results": None}


def _split16(a):
    hi = a.astype(np.float16)
    lo = (a - hi.astype(np.float64)).astype(np.float16)
    return hi, lo


def _build_host_inputs(w1, b1, w2, b2, w3, b3):
    """Small host-side layout prep of the SIREN weights (fp64 for exactness)."""
    w1 = np.asarray(w1, np.float64)  # [32, 1]
    b1 = np.asarray(b1, np.float64)  # [32]
    w2 = np.asarray(w2, np.float64)  # [32, 32]
    b2 = np.asarray(b2, np.float64)  # [32]
    w3 = np.asarray(w3, np.float64)  # [512, 32]
    b3 = np.asarray(b3, np.float64)  # [512]

    t = np.linspace(-1.0, 1.0, L)
    tr = -t  # reversed t
    th, tl = _split16(tr)
    t4 = np.stack([th, tl, th, tl]).astype(np.float16)          # [4, L]

    w1s = OMEGA0 * w1[:, 0]                                      # [32]
    wh, wl = _split16(w1s)
    a1 = np.stack([wh, wh, wl, wl]).astype(np.float16)           # [4, 32]
    # pairing: (wh*th) + (wh*tl) + (wl*th) + (wl*tl) = w1s * tr (to ~2^-22)

    b1rep = np.tile((OMEGA0 * b1).astype(np.float32), 4)[:, None]   # [128,1]
    a2 = np.tile((OMEGA0 * w2.T).astype(np.float16), (4, 1))     # [128, 32]
    b2rep = np.tile((OMEGA0 * b2).astype(np.float32), 4)[:, None]   # [128,1]

    # W3T[c, 32*i+o] = w3[16*o+i, c]; row 32 = b3[16*o+i]
    w3t = np.zeros((33, 512), np.float16)
    oi = np.arange(512)
    o, i = oi // CIN, oi % CIN
    f = 32 * i + o
    w3t[:32, f] = w3[oi, :].T.astype(np.float16)
    w3t[32, f] = b3[oi].astype(np.float16)
    ones_row = np.ones((1, L), np.float16)
    return dict(t4=t4, a1=a1, b1rep=b1rep, a2=a2, b2rep=b2rep, w3t=w3t,
                ones_row=ones_row)


def _conv_tasks():
    """(p, t, i) task list and its round-robin split over 4 PE col groups."""
    tasks = []
    for p in range(4):
        for t in range(4 * p + 4):
            for i in range(CIN):
                tasks.append((p, t, i))
    groups = [[], [], [], []]
    for k, task in enumerate(tasks):
        groups[k % 4].append(task)
    return groups


def _gen():
    import concourse.bass as bass
    import concourse.mybir as mybir
    import concourse.tile as tile
    from concourse import bacc

    F32 = mybir.dt.float32
    F16 = mybir.dt.float16
    AF = mybir.ActivationFunctionType
    OP = mybir.AluOpType

    nc = bacc.Bacc()
    xpad = nc.dram_tensor("xpad", [CIN, XSW], F16, kind="ExternalInput")
    t4 = nc.dram_tensor("t4", [4, L], F16, kind="ExternalInput")
    a1 = nc.dram_tensor("a1", [4, 32], F16, kind="ExternalInput")
    b1rep = nc.dram_tensor("b1rep", [128, 1], F32, kind="ExternalInput")
    a2 = nc.dram_tensor("a2", [128, 32], F16, kind="ExternalInput")
    b2rep = nc.dram_tensor("b2rep", [128, 1], F32, kind="ExternalInput")
    w3t = nc.dram_tensor("w3t", [33, 512], F16, kind="ExternalInput")
    ones_row = nc.dram_tensor("ones_row", [1, L], F16, kind="ExternalInput")
    cbias = nc.dram_tensor("cbias", [32, 1], F32, kind="ExternalInput")
    out_res = nc.dram_tensor("out_res", [32, L], F32, kind="ExternalOutput")

    groups = _conv_tasks()

    with tile.TileContext(nc) as tc:
        with tc.tile_pool(name="pool", bufs=1) as pool, \
             tc.tile_pool(name="pps", bufs=1, space="PSUM") as pps:

            # ---------- load small inputs ----------
            t4t = pool.tile([4, L], F16)
            nc.sync.dma_start(t4t[:], t4[:, :])
            a1t = pool.tile([4, 32], F16)
            nc.sync.dma_start(a1t[:], a1[:, :])
            b1t = pool.tile([128, 1], F32)
            nc.sync.dma_start(b1t[:], b1rep[:, :])
            a2t = pool.tile([128, 32], F16)
            nc.sync.dma_start(a2t[:], a2[:, :])
            b2t = pool.tile([128, 1], F32)
            nc.sync.dma_start(b2t[:], b2rep[:, :])
            w3tt = pool.tile([33, 512], F16)
            nc.sync.dma_start(w3tt[:], w3t[:, :])
            cbt = pool.tile([32, 1], F32)
            nc.sync.dma_start(cbt[:], cbias[:, :])

            # ---------- XS bank build: 4 chains of 4 i's each ----------
            NG = 4       # i's per group
            GW = NG * XSW
            xss = [pool.tile([128, GW], F16, name=f"xs_{gg}", tag=f"xs{gg}")
                   for gg in range(4)]
            for gg in range(4):
                xs3 = xss[gg].rearrange("p (i c) -> p i c", i=NG)
                nc.sync.dma_start(xs3[0:1, :, :], xpad[NG * gg:NG * gg + NG, :])
                nc.vector.memset(xs3[:, :, 0:128], 0.0)
                for k in range(7):
                    n = 1 << k
                    nc.sync.dma_start(xs3[n:2 * n, :, n:XSW],
                                      xs3[0:n, :, 0:XSW - n])

            # ---------- SIREN L1 (stacked [128,512]) ----------
            ps1 = pps.tile([128, 512], F32)
            for a in range(4):
                nc.tensor.matmul(ps1[32 * a:32 * a + 32, :],
                                 a1t[:, :],
                                 t4t[:, 512 * a:512 * a + 512],
                                 start=True, stop=True,
                                 tile_position=(0, 32 * a))
            w_t = pool.tile([128, 512], F32)
            nc.vector.tensor_scalar(w_t[:], ps1[:], b1t[:], INV_2PI,
                                    OP.add, OP.mult)
            u_t = pool.tile([128, 512], F32)
            nc.vector.tensor_scalar(u_t[:], w_t[:], MAGIC, None, OP.add)
            n_t = pool.tile([128, 512], F32)
            nc.vector.tensor_scalar(n_t[:], u_t[:], MAGIC, None, OP.subtract)
            d_t = pool.tile([128, 512], F32)
            nc.vector.tensor_tensor(d_t[:], w_t[:], n_t[:], OP.subtract)
            h1 = pool.tile([128, 512], F16)
            nc.scalar.activation(h1[:], d_t[:], AF.Sin, scale=TWO_PI)

            # ---------- SIREN L2 ----------
            ps2 = pps.tile([128, 512], F32)
            for a in range(4):
                nc.tensor.matmul(ps2[32 * a:32 * a + 32, :],
                                 a2t[32 * a:32 * a + 32, :],
                                 h1[32 * a:32 * a + 32, :],
                                 start=True, stop=True,
                                 tile_position=(32 * a, 32 * a))
            w2_t = pool.tile([128, 512], F32)
            nc.vector.tensor_scalar(w2_t[:], ps2[:], b2t[:], INV_2PI,
                                    OP.add, OP.mult)
            u2_t = pool.tile([128, 512], F32)
            nc.vector.tensor_scalar(u2_t[:], w2_t[:], MAGIC, None, OP.add)
            n2_t = pool.tile([128, 512], F32)
            nc.vector.tensor_scalar(n2_t[:], u2_t[:], MAGIC, None, OP.subtract)
            d2_t = pool.tile([128, 512], F32)
            nc.vector.tensor_tensor(d2_t[:], w2_t[:], n2_t[:], OP.subtract)
            # H2 flat [33, 2048]: rows 0-31 features, row 32 ones
            h2 = pool.tile([33, L], F16)
            nc.sync.dma_start(h2[32:33, :], ones_row[:, :])
            for a in range(4):
                nc.scalar.activation(h2[0:32, 512 * a:512 * a + 512],
                                     d2_t[32 * a:32 * a + 32, :],
                                     AF.Sin, scale=TWO_PI)

            # ---------- SIREN L3 + Conv ----------
            # Slots of 4 tasks (one per PE col group) MUST share one psum
            # bank (same p): concurrent col-tiled matmuls writing different
            # banks corrupt results. Per p, tasks ordered (t, i); slots
            # round-robin over p so small-t work comes first; L3 blocks are
            # emitted just-in-time before the conv slots that need them.
            k16 = pool.tile([128, 16 * 512], F16)
            accs = []
            for p in range(4):
                acc = pps.tile([128, 512], F32, name=f"acc_{p}", tag=f"acc{p}")
                accs.append(acc)
            ptasks = {p: [(t, i) for t in range(4 * p + 4) for i in range(CIN)]
                      for p in range(4)}
            slots = []  # each: (p, [(g, t, i) x4], max_t)
            pos = {p: 0 for p in range(4)}
            while any(pos[p] < len(ptasks[p]) for p in range(4)):
                for p in range(4):
                    if pos[p] < len(ptasks[p]):
                        four = ptasks[p][pos[p]:pos[p] + 4]
                        pos[p] += 4
                        slots.append((p, [(g, t, i) for g, (t, i) in enumerate(four)],
                                      max(t for t, _ in four)))
            # start/stop bookkeeping per (p, g)
            last_touch = {}
            for si, (p, four, _) in enumerate(slots):
                for g, t, i in four:
                    last_touch[(p, g)] = (si, g)
            started = set()
            sptr = 0
            for th in range(16):
                ps3 = pps.tile([128, 512], F32, name=f"ps3_{th}", tag="ps3", bufs=2)
                nc.tensor.matmul(ps3[:, :],
                                 h2[:, 128 * th:128 * th + 128],
                                 w3tt[:, :],
                                 start=True, stop=True)
                nc.vector.tensor_copy(k16[:, 512 * th:512 * th + 512], ps3[:, :])
                while sptr < len(slots) and slots[sptr][2] <= th:
                    p, four, _ = slots[sptr]
                    for g, t, i in four:
                        first = (p, g) not in started
                        started.add((p, g))
                        last = last_touch[(p, g)] == (sptr, g)
                        xs_g = xss[i // 4]
                        col = XSW * (i % 4) + PAD + 512 * p - 128 * t
                        nc.tensor.matmul(
                            accs[p][32 * g:32 * g + 32, :],
                            k16[:, 512 * t + 32 * i: 512 * t + 32 * i + 32],
                            xs_g[:, col:col + 512],
                            start=first, stop=last,
                            tile_position=(0, 32 * g))
                    sptr += 1
            assert sptr == len(slots), (sptr, len(slots))

            # ---------- reduce col groups + bias, write out ----------
            for p in range(4):
                sb = pool.tile([32, 512], F32, name=f"sb_{p}", tag="sbout", bufs=2)
                nc.vector.tensor_scalar(sb[:], accs[p][0:32, :], cbt[:], None,
                                        OP.add)
                for g in range(1, 4):
                    nc.vector.tensor_tensor(sb[:], sb[:],
                                            accs[p][32 * g:32 * g + 32, :],
                                            OP.add)
                nc.sync.dma_start(out_res[:, 512 * p:512 * p + 512], sb[:])

    nc.finalize()
    return nc


def _get_runner():
    """Build (once) a cached jitted shard_map runner for the 8-core SPMD kernel."""
    if "runner" in _COMPILED:
        return _COMPILED["runner"]

    import jax
    import numpy as np_
    from jax.sharding import Mesh, PartitionSpec
    from jax.experimental.shard_map import shard_map
    import concourse.mybir as mybir
    from concourse import bass2jax
    from concourse.bass2jax import _bass_exec_p, install_neuronx_cc_hook

    if "nc" not in _COMPILED:
        _COMPILED["nc"] = _gen()
    nc = _COMPILED["nc"]

    install_neuronx_cc_hook()

    partition_name = nc.partition_id_tensor.name if nc.partition_id_tensor else None
    in_names, out_names, out_avals, zero_outs = [], [], [], []
    for alloc in nc.m.functions[0].allocations:
        if not isinstance(alloc, mybir.MemoryLocationSet):
            continue
        name = alloc.memorylocations[0].name
        if alloc.kind == "ExternalInput":
            if name != partition_name:
                in_names.append(name)
        elif alloc.kind == "ExternalOutput":
            out_names.append(name)
            shape = tuple(alloc.tensor_shape)
            dtype = mybir.dt.np(alloc.dtype)
            out_avals.append(jax.core.ShapedArray(shape, dtype))
            zero_outs.append(np.zeros(shape, dtype))
    n_params = len(in_names)
    n_outs = len(out_avals)
    all_in_names = list(in_names) + list(out_names)
    if partition_name is not None:
        all_in_names.append(partition_name)
    donate = tuple(range(n_params, n_params + n_outs))

    def _body(*args):
        operands = list(args)
        if partition_name is not None:
            operands.append(bass2jax.partition_id_tensor())
        outs = _bass_exec_p.bind(
            *operands,
            out_avals=tuple(out_avals),
            in_names=tuple(all_in_names),
            out_names=tuple(out_names),
            lowering_input_output_aliases=(),
            sim_require_finite=True,
            sim_require_nnan=True,
            nc=nc,
        )
        return tuple(outs)

    devices = jax.devices()[:B]
    mesh = Mesh(np.asarray(devices, dtype=object), ("core",))
    in_specs = (PartitionSpec("core"),) * (n_params + n_outs)
    out_specs = (PartitionSpec("core"),) * len(out_names)
    sharded = jax.jit(
        shard_map(_body, mesh=mesh, in_specs=in_specs, out_specs=out_specs,
                  check_rep=False),
        donate_argnums=donate, keep_unused=True,
    )

    runner = dict(sharded=sharded, in_names=in_names, out_names=out_names,
                  out_avals=out_avals, zero_outs=zero_outs)
    _COMPILED["runner"] = runner
    return runner


def _run_spmd(in_maps):
    import numpy as np_
    r = _get_runner()
    n_cores = len(in_maps)
    per_core = [[np.asarray(m[name]) for name in r["in_names"]] for m in in_maps]
    concat_in = [np.concatenate([per_core[c][i] for c in range(n_cores)], axis=0)
                 for i in range(len(r["in_names"]))]
    concat_zeros = [np.zeros((n_cores * z.shape[0], *z.shape[1:]), z.dtype)
                    for z in r["zero_outs"]]
    out_arrs = r["sharded"](*concat_in, *concat_zeros)
    out_arrs = [np.asarray(a) for a in out_arrs]
    return [
        {name: out_arrs[i].reshape(n_cores, *r["out_avals"][i].shape)[c]
         for i, name in enumerate(r["out_names"])}
        for c in range(n_cores)
    ]


def _make_in_maps(x, conv_bias, host):
    cb = np.asarray(conv_bias, np.float32).reshape(32, 1)
    in_maps = []
    for b in range(B):
        xpad = np.zeros((CIN, XSW), np.float16)
        xpad[:, PAD:PAD + L] = x[b].astype(np.float16)
        in_maps.append(dict(xpad=xpad, cbias=cb, **host))
    return in_maps


def _postprocess(results):
    out = np.zeros((B, COUT, L), np.float32)
    for b in range(B):
        out[b] = results[b]["out_res"]
    return out


def kernel(x, w1, b1, w2, b2, w3, b3, conv_bias):
    x = np.asarray(x)
    host = _build_host_inputs(w1, b1, w2, b2, w3, b3)
    in_maps = _make_in_maps(x, conv_bias, host)
    results = _run_spmd(in_maps)
    return _postprocess(results)



# revision 15
# speedup vs baseline: 502.3217x; 502.3217x over previous
"""CKConv Trainium2 kernel.

Math (derived from the reference):
  out[b,o,l] = sum_i sum_{d=0}^{l} g[o,i,d] * x[b,i,l-d] + conv_bias[o]
  g[o,i,d]   = k_full[o,i,2047-d],  k_full = w3 @ h2 + b3
  h2 = sin(30*(w2 @ h1 + b2)), h1 = sin(30*(w1 @ t + b1)), t = linspace(-1,1,L)

Forward formulation: enumerate kernel taps by the FORWARD index j = 2047-d.
  out[o,l] = sum_i sum_j k_full[16o+i, j] * x[i, l + j - 2047]
With j = 128*th + q (th = tap block, q = psum-contraction lane):
  out[o, 512p+c] += sum_q k16[q, 512*th+32i+o] * XT[q, i, (512p+128th-1920)+c]
where
  k16 block th = H2f[:,128th:+128]^T @ W3T (SIREN on forward t, ones row -> b3),
  XT[q, i, T] = xpadF[i, T + q] = x[i, T+q-127], xpadF = [127 zeros | x | 1 zero].
XT is built by ONE DMA per input channel whose source AP is the all-positive
diagonal [(1,128),(1,XSW3)] — each partition q reads a 2-byte-shifted window.
Causal zero-prefix columns (c < 1920-512p-128th, zero for every q) are trimmed
from each matmul, cutting streamed PE columns 327680 -> 278528.

Per core (data-parallel over batch b):
  - SIREN computed per 512-column time block in order a=3,2,1,0; k16 blocks
    th=15..0 emitted just-in-time, so output block p=0 (which needs only
    th>=12) finishes first and its psum reduce overlaps the remaining conv.
  - Conv: 640 matmuls [K=128, M=32, N=512] fp16 over 4 PE column groups
    (tile_position) with per-p psum accumulators; block p needs th >= 12-4p.
  - Col groups reduced on DVE, + conv_bias, written out as fp16 [32, 2048];
    host casts to fp32.
"""
import hashlib
import numpy as np

OMEGA0 = 30.0
CIN, COUT, HID = 16, 32, 32
B, L = 8, 2048
XSW3 = 2048        # per-i XS row width (cols 0..383 of the old bank are never read)
XP3 = 2176         # xpadF row width: 127 zeros | 2048 x | 1 zero
PI = float(np.pi)
TWO_PI = float(2 * np.pi)
MAGIC = 12582912.0  # 1.5 * 2**23, fp32 round-to-nearest trick
INV_2PI = float(1.0 / (2 * np.pi))

_COMPILED = {}


def _split16(a):
    hi = a.astype(np.float16)
    lo = (a - hi.astype(np.float64)).astype(np.float16)
    return hi, lo


def _build_host_inputs(w1, b1, w2, b2, w3, b3):
    """Small host-side layout prep of the SIREN weights (fp64 for exactness)."""
    w1 = np.asarray(w1, np.float64)  # [32, 1]
    b1 = np.asarray(b1, np.float64)  # [32]
    w2 = np.asarray(w2, np.float64)  # [32, 32]
    b2 = np.asarray(b2, np.float64)  # [32]
    w3 = np.asarray(w3, np.float64)  # [512, 32]
    b3 = np.asarray(b3, np.float64)  # [512]

    t = np.linspace(-1.0, 1.0, L)    # forward time grid
    th, tl = _split16(t)
    t4 = np.stack([th, tl, th, tl]).astype(np.float16)          # [4, L]

    w1s = OMEGA0 * w1[:, 0]                                      # [32]
    wh, wl = _split16(w1s)
    a1 = np.stack([wh, wh, wl, wl]).astype(np.float16)           # [4, 32]
    # pairing: (wh*th) + (wh*tl) + (wl*th) + (wl*tl) = w1s * t (to ~2^-22)

    b1rep = np.tile((OMEGA0 * b1).astype(np.float32), 4)[:, None]   # [128,1]
    a2 = np.tile((OMEGA0 * w2.T).astype(np.float16), (4, 1))     # [128, 32]
    b2rep = np.tile((OMEGA0 * b2).astype(np.float32), 4)[:, None]   # [128,1]

    # W3T[c, 32*i+o] = w3[16*o+i, c]; row 32 = b3[16*o+i]
    w3t = np.zeros((33, 512), np.float16)
    oi = np.arange(512)
    o, i = oi // CIN, oi % CIN
    f = 32 * i + o
    w3t[:32, f] = w3[oi, :].T.astype(np.float16)
    w3t[32, f] = b3[oi].astype(np.float16)
    ones_row = np.ones((1, L), np.float16)
    return dict(t4=t4, a1=a1, b1rep=b1rep, a2=a2, b2rep=b2rep, w3t=w3t,
                ones_row=ones_row)


def _gen():
    import concourse.bass as bass
    import concourse.mybir as mybir
    import concourse.tile as tile
    from concourse import bacc
    from concourse.ap import AP

    F32 = mybir.dt.float32
    F16 = mybir.dt.float16
    AF = mybir.ActivationFunctionType
    OP = mybir.AluOpType

    nc = bacc.Bacc()
    xpadF = nc.dram_tensor("xpadF", [CIN, XP3], F16, kind="ExternalInput")
    t4 = nc.dram_tensor("t4", [4, L], F16, kind="ExternalInput")
    a1 = nc.dram_tensor("a1", [4, 32], F16, kind="ExternalInput")
    b1rep = nc.dram_tensor("b1rep", [128, 1], F32, kind="ExternalInput")
    a2 = nc.dram_tensor("a2", [128, 32], F16, kind="ExternalInput")
    b2rep = nc.dram_tensor("b2rep", [128, 1], F32, kind="ExternalInput")
    w3t = nc.dram_tensor("w3t", [33, 512], F16, kind="ExternalInput")
    ones_row = nc.dram_tensor("ones_row", [1, L], F16, kind="ExternalInput")
    cbias = nc.dram_tensor("cbias", [32, 1], F32, kind="ExternalInput")
    out_res = nc.dram_tensor("out_res", [32, L], F16, kind="ExternalOutput")

    with tile.TileContext(nc) as tc:
        with tc.tile_pool(name="pool", bufs=1) as pool, \
             tc.tile_pool(name="pps", bufs=1, space="PSUM") as pps:

            # ---------- load small inputs ----------
            t4t = pool.tile([4, L], F16)
            nc.sync.dma_start(t4t[:], t4[:, :])
            a1t = pool.tile([4, 32], F16)
            nc.sync.dma_start(a1t[:], a1[:, :])
            b1t = pool.tile([128, 1], F32)
            nc.sync.dma_start(b1t[:], b1rep[:, :])
            a2t = pool.tile([128, 32], F16)
            nc.sync.dma_start(a2t[:], a2[:, :])
            b2t = pool.tile([128, 1], F32)
            nc.sync.dma_start(b2t[:], b2rep[:, :])
            w3tt = pool.tile([33, 512], F16)
            nc.sync.dma_start(w3tt[:], w3t[:, :])
            cbt = pool.tile([32, 1], F32)
            nc.sync.dma_start(cbt[:], cbias[:, :])

            # ---------- XT bank: diagonal build, one DMA per channel ----------
            # XT[q, i, T] = xpadF[4u+i, T + q]. Channels 0-5 on the SP queue,
            # 6-11 on the gpsimd SWDGE queue (both start immediately); 12-15
            # go on the Activation queue but are emitted after the critical
            # a=3 SIREN activations so they don't delay the conv head.
            NG = 4       # i's per tile
            GW = NG * XSW3
            xss = [pool.tile([128, GW], F16, name=f"xs_{u}", tag=f"xs{u}")
                   for u in range(4)]

            def _xs_dma(i, eng):
                u, il = i // NG, i % NG
                xs3 = xss[u].rearrange("p (i c) -> p i c", i=NG)
                src = AP(xpadF, i * XP3, [(1, 128), (1, XSW3)])
                eng.dma_start(xs3[:, il, :], src)

            for i in range(0, 6):
                _xs_dma(i, nc.sync)
            for i in range(6, 12):
                _xs_dma(i, nc.gpsimd)

            # preload the Sin LUT so the first real activation skips the
            # table-load stall
            warm = pool.tile([1, 1], F32)
            nc.scalar.activation(warm[:], cbt[0:1, 0:1], AF.Sin, scale=1.0)

            # ---------- SIREN L1 matmuls (all blocks up front) ----------
            ps1 = pps.tile([128, 512], F32)
            for a in (3, 2, 1, 0):
                nc.tensor.matmul(ps1[32 * a:32 * a + 32, :],
                                 a1t[:, :],
                                 t4t[:, 512 * a:512 * a + 512],
                                 start=True, stop=True,
                                 tile_position=(0, 32 * a))

            # shared SBUF scratch, sliced per block a
            w_t = pool.tile([128, 512], F32)
            u_t = pool.tile([128, 512], F32)
            n_t = pool.tile([128, 512], F32)
            d_t = pool.tile([128, 512], F32)
            h1 = pool.tile([128, 512], F16)
            ps2 = pps.tile([128, 512], F32)
            w2_t = pool.tile([128, 512], F32)
            u2_t = pool.tile([128, 512], F32)
            n2_t = pool.tile([128, 512], F32)
            d2_t = pool.tile([128, 512], F32)
            # H2 flat [33, 2048]: rows 0-31 features, row 32 ones
            h2 = pool.tile([33, L], F16)
            nc.sync.dma_start(h2[32:33, :], ones_row[:, :])

            k16 = pool.tile([128, 16 * 512], F16)
            accs = []
            for p in range(4):
                acc = pps.tile([128, 512], F32, name=f"acc_{p}", tag=f"acc{p}")
                accs.append(acc)

            # conv task activation: output block p needs tap blocks th >= 12-4p
            def active_ps(th):
                return [p for p in range(4) if th >= 12 - 4 * p]

            # ---------- per-block SIREN (a=3..0) + JIT L3 + conv ----------
            for a in (3, 2, 1, 0):
                s = slice(32 * a, 32 * a + 32)
                # L1 epilogue: magic-number range reduction + Sin LUT
                nc.vector.tensor_scalar(w_t[s, :], ps1[s, :], b1t[s, :],
                                        INV_2PI, OP.add, OP.mult)
                nc.vector.tensor_scalar(u_t[s, :], w_t[s, :], MAGIC, None, OP.add)
                nc.vector.tensor_scalar(n_t[s, :], u_t[s, :], MAGIC, None,
                                        OP.subtract)
                nc.vector.tensor_tensor(d_t[s, :], w_t[s, :], n_t[s, :],
                                        OP.subtract)
                nc.scalar.activation(h1[s, :], d_t[s, :], AF.Sin, scale=TWO_PI)
                # L2
                nc.tensor.matmul(ps2[s, :], a2t[s, :], h1[s, :],
                                 start=True, stop=True,
                                 tile_position=(32 * a, 32 * a))
                nc.vector.tensor_scalar(w2_t[s, :], ps2[s, :], b2t[s, :],
                                        INV_2PI, OP.add, OP.mult)
                nc.vector.tensor_scalar(u2_t[s, :], w2_t[s, :], MAGIC, None,
                                        OP.add)
                nc.vector.tensor_scalar(n2_t[s, :], u2_t[s, :], MAGIC, None,
                                        OP.subtract)
                nc.vector.tensor_tensor(d2_t[s, :], w2_t[s, :], n2_t[s, :],
                                        OP.subtract)
                nc.scalar.activation(h2[0:32, 512 * a:512 * a + 512],
                                     d2_t[s, :], AF.Sin, scale=TWO_PI)
                if a == 3:
                    for i in range(12, CIN):
                        _xs_dma(i, nc.scalar)

                # L3 blocks th = 4a+3 .. 4a (descending)
                for th in range(4 * a + 3, 4 * a - 1, -1):
                    ps3 = pps.tile([128, 512], F32, name=f"ps3_{th}",
                                   tag="ps3", bufs=2)
                    nc.tensor.matmul(ps3[:, :],
                                     h2[:, 128 * th:128 * th + 128],
                                     w3tt[:, :],
                                     start=True, stop=True)
                    nc.vector.tensor_copy(k16[:, 512 * th:512 * th + 512],
                                          ps3[:, :])

            # ---------- conv, u-outer so channel-group u is first needed at
            # ~u/4 of the stream, matching XS DMA delivery order ----------
            for u in range(4):
                for th in range(15, -1, -1):
                    for p in active_ps(th):
                        z = max(0, 1920 - 512 * p - 128 * th)
                        t0 = 512 * p + 128 * th + z - 1920
                        for g in range(4):
                            i = 4 * u + g
                            col = XSW3 * g + t0
                            nc.tensor.matmul(
                                accs[p][32 * g:32 * g + 32, z:512],
                                k16[:, 512 * th + 32 * i:
                                    512 * th + 32 * i + 32],
                                xss[u][:, col:col + 512 - z],
                                start=(th == 15 and u == 0),
                                stop=(th == 12 - 4 * p and u == 3),
                                tile_position=(0, 32 * g))
                    # reduce + write out any p whose accumulation just closed
                    # (DVE may read at most one PSUM operand per instruction,
                    # so chain through SBUF)
                    if u == 3:
                        for p in range(4):
                            if th == 12 - 4 * p:
                                s1 = pool.tile([32, 512], F32, name=f"s1_{p}",
                                               tag="sred1", bufs=2)
                                nc.vector.tensor_scalar(s1[:], accs[p][0:32, :],
                                                        cbt[:], None, OP.add)
                                nc.vector.tensor_tensor(s1[:], s1[:],
                                                        accs[p][32:64, :],
                                                        OP.add)
                                nc.vector.tensor_tensor(s1[:], s1[:],
                                                        accs[p][64:96, :],
                                                        OP.add)
                                sb = pool.tile([32, 512], F16, name=f"sb_{p}",
                                               tag="sbout", bufs=2)
                                nc.vector.tensor_tensor(sb[:], s1[:],
                                                        accs[p][96:128, :],
                                                        OP.add)
                                nc.sync.dma_start(
                                    out_res[:, 512 * p:512 * p + 512], sb[:])

    nc.finalize()
    return nc


def _get_runner():
    """Build (once) a cached jitted shard_map runner for the 8-core SPMD kernel.

    No donation: output operands are plain (cached, device-resident) zero
    tensors reused every call; the kernel writes every element of out_res.
    """
    if "runner" in _COMPILED:
        return _COMPILED["runner"]

    import jax
    import warnings
    from jax.sharding import Mesh, PartitionSpec, NamedSharding
    with warnings.catch_warnings():
        warnings.simplefilter("ignore")
        from jax.experimental.shard_map import shard_map
    import concourse.mybir as mybir
    from concourse import bass2jax
    from concourse.bass2jax import _bass_exec_p, install_neuronx_cc_hook

    if "nc" not in _COMPILED:
        _COMPILED["nc"] = _gen()
    nc = _COMPILED["nc"]

    install_neuronx_cc_hook()

    partition_name = nc.partition_id_tensor.name if nc.partition_id_tensor else None
    in_names, out_names, out_avals, zero_outs = [], [], [], []
    for alloc in nc.m.functions[0].allocations:
        if not isinstance(alloc, mybir.MemoryLocationSet):
            continue
        name = alloc.memorylocations[0].name
        if alloc.kind == "ExternalInput":
            if name != partition_name:
                in_names.append(name)
        elif alloc.kind == "ExternalOutput":
            out_names.append(name)
            shape = tuple(alloc.tensor_shape)
            dtype = mybir.dt.np(alloc.dtype)
            out_avals.append(jax.core.ShapedArray(shape, dtype))
            zero_outs.append(np.zeros(shape, dtype))
    n_params = len(in_names)
    n_outs = len(out_avals)
    all_in_names = list(in_names) + list(out_names)
    if partition_name is not None:
        all_in_names.append(partition_name)

    def _body(*args):
        operands = list(args)
        if partition_name is not None:
            operands.append(bass2jax.partition_id_tensor())
        outs = _bass_exec_p.bind(
            *operands,
            out_avals=tuple(out_avals),
            in_names=tuple(all_in_names),
            out_names=tuple(out_names),
            lowering_input_output_aliases=(),
            sim_require_finite=True,
            sim_require_nnan=True,
            nc=nc,
        )
        return tuple(outs)

    devices = jax.devices()[:B]
    mesh = Mesh(np.asarray(devices, dtype=object), ("core",))
    in_specs = (PartitionSpec("core"),) * (n_params + n_outs)
    out_specs = (PartitionSpec("core"),) * len(out_names)
    sharded = jax.jit(
        shard_map(_body, mesh=mesh, in_specs=in_specs, out_specs=out_specs,
                  check_rep=False),
        keep_unused=True,
    )
    sharding = NamedSharding(mesh, PartitionSpec("core"))

    def _fast_compiled(args):
        """C++ fast-path Compiled for the steady-state loop (lazy, cached).

        A fresh jit must be traced inside fast_dispatch_compile — reusing
        `sharded` would return the cached effectful jaxpr.
        """
        if "fast" not in _COMPILED:
            from concourse.bass2jax import fast_dispatch_compile
            fresh = jax.jit(
                shard_map(_body, mesh=mesh, in_specs=in_specs,
                          out_specs=out_specs, check_rep=False),
                keep_unused=True,
            )
            _COMPILED["fast"] = fast_dispatch_compile(
                lambda: fresh.lower(*args).compile())
        return _COMPILED["fast"]

    runner = dict(sharded=sharded, in_names=in_names, out_names=out_names,
                  out_avals=out_avals, zero_outs=zero_outs, mesh=mesh,
                  sharding=sharding, n_params=n_params,
                  partition_name=partition_name, body=_body,
                  all_in_names=all_in_names, nc=nc,
                  fast_compiled=_fast_compiled)
    _COMPILED["runner"] = runner
    return runner


def _device_args(in_maps):
    """Device-resident operand list for the jitted runner.

    Static (weight-derived) inputs and the output-operand zeros are uploaded
    once and cached (keyed on content); the per-batch xpadF is uploaded fresh
    on every call.
    """
    import jax
    r = _get_runner()
    n_cores = len(in_maps)
    sh = r["sharding"]

    statics = [n for n in r["in_names"] if n != "xpadF"]
    hsh = hashlib.blake2b(digest_size=16)
    for name in statics:
        for m in in_maps:
            hsh.update(np.ascontiguousarray(m[name]).tobytes())
    key = hsh.hexdigest()
    if _COMPILED.get("static_key") != key:
        dev = {}
        for name in statics:
            cat = np.concatenate([np.asarray(m[name]) for m in in_maps], axis=0)
            dev[name] = jax.device_put(cat, sh)
        zeros = [jax.device_put(
                    np.zeros((n_cores * z.shape[0], *z.shape[1:]), z.dtype), sh)
                 for z in r["zero_outs"]]
        jax.block_until_ready(list(dev.values()) + zeros)
        _COMPILED["static_dev"] = dev
        _COMPILED["zero_dev"] = zeros
        _COMPILED["static_key"] = key

    xcat = np.concatenate([np.asarray(m["xpadF"]) for m in in_maps], axis=0)
    dx = jax.device_put(xcat, sh)
    args = [dx if n == "xpadF" else _COMPILED["static_dev"][n]
            for n in r["in_names"]]
    return args + list(_COMPILED["zero_dev"])


def _run_spmd(in_maps):
    r = _get_runner()
    n_cores = len(in_maps)
    args = _device_args(in_maps)
    out = r["fast_compiled"](args)(*args)
    try:
        out[0].copy_to_host_async()
    except Exception:
        pass
    arrs = [np.asarray(a) for a in out]
    return [
        {name: arrs[i].reshape(n_cores, *r["out_avals"][i].shape)[c]
         for i, name in enumerate(r["out_names"])}
        for c in range(n_cores)
    ]


def _make_in_maps(x, conv_bias, host):
    cb = np.asarray(conv_bias, np.float32).reshape(32, 1)
    in_maps = []
    for b in range(B):
        xpadF = np.zeros((CIN, XP3), np.float16)
        xpadF[:, 127:127 + L] = x[b].astype(np.float16)
        in_maps.append(dict(xpadF=xpadF, cbias=cb, **host))
    return in_maps


def _postprocess(results):
    out = np.zeros((B, COUT, L), np.float32)
    for b in range(B):
        out[b] = results[b]["out_res"].astype(np.float32)
    return out


def kernel(x, w1, b1, w2, b2, w3, b3, conv_bias):
    x = np.asarray(x)
    host = _build_host_inputs(w1, b1, w2, b2, w3, b3)
    in_maps = _make_in_maps(x, conv_bias, host)
    results = _run_spmd(in_maps)
    return _postprocess(results)
